# revision 21
# baseline (speedup 1.0000x reference)
"""Cosine-similarity 1-NN over 1M x 256 f32 embeddings on 8 TRN2 NeuronCores.

v3, fp8 DoubleRow streaming: the kernel is a pure HBM-bandwidth problem, so
the device-side table is stored fp8 e4m3 (quarter of f32 traffic), and the
TensorEngine's DoubleRow perf mode virtualizes the PE array to a 128x256
contraction — both 128-dim chunks of each row contract in ONE matmul at one
row/cycle. Candidate selection only needs the true argmax to survive into a
top-8-per-partition candidate set that the host rescores exactly in f64;
fp8 perturbs dots by sigma ~0.5 (at qx16 scaling) against partition-level
top-8 margins of ~30, so ranking by fp8 dots is safe (verified in emulation
vs the reference argmax: the true best ranks #1 in its partition, 75 vs 41
for the 8th-best).

Host-side prep (one-time, outside the timed NEFF): table -> [128, 2, N]
fp8 (dim d of chunk c at [d, c, row]), q -> qhat * 16 cast fp8 (scaling
centers q's entries in e4m3's dynamic range; dots scale by 16, ranking
unchanged). Cores 0-6 take 126976 rows, core 7 the rest zero-padded.

Per-core graph, rows_pc = 126976 = 16 tiles x 7936 rows:
  - et tile [128, 2, 7936] fp8: both chunk loads on the SP (sync) HWDGE
    ring (~1 MB per DMA, 2 MB per tile). Keeping the SP queue DMA-only
    means a blocked buffer-free wait never head-of-line-blocks compute
    work (this alone was worth ~40 us vs mixing loads onto the ACT ring).
  - 16 matmuls per tile: lhsT = q3[:, :, 0:1] ([128, 2, 1] fp8), rhs =
    et[:, :, g*496:(g+1)*496] ([128, 2, 496]), perf_mode=DoubleRow ->
    dots [1, 496] f32 in PSUM, one instruction per group.
  - Evacuation alternates ACT/DVE copies into a [1, 7936] f32 stage row,
    then one ACT-ring SBUF->SBUF DMA reshapes to dots[:, t*62:(t+1)*62].
  - Epilogue: per-partition top-8 (vector.max / max_index).
Steady-state: 92 us/scan on HW = the ~358 GB/s per-core HBM roofline for
32.5 MB/core (TimelineSim floor for the DMA stream alone: 91.8 us).

Host maps (partition p, col c) -> local row (c//62)*7936 + p*62 + c%62 and
rescores all candidates exactly.
"""
import numpy as np
import ml_dtypes
from contextlib import ExitStack

from concourse import bacc, tile, mybir
from concourse.bass_utils import run_bass_kernel_spmd

EPS = 1e-8
P = 128
D = 256
N_CORES = 8
N_ROWS = 1000000

G = 496            # dots per PSUM group (<= 512 f32 / one 2KB PSUM bank)
NG = 16            # PSUM groups per full tile
NT = G * NG        # 7936 rows per full tile (= 62 * 128)
T = 15             # full tiles per core
NT_L = 6016        # tail tile: 47*128 rows (12 groups of 496 + one of 64)
ROWS_PC = NT * T + NT_L   # 125056 = 977*128: minimal multiple of 128 with
                          # 8 * ROWS_PC >= 1M (1.5% less DMA than padding
                          # to a uniform 16th tile)
CPT = NT // P      # 62 dot columns per full tile
CPT_L = NT_L // P  # 47 dot columns in the tail tile
CC = T * CPT + CPT_L      # 977 dot columns per partition

FP8 = ml_dtypes.float8_e4m3
Q_SCALE = 16.0


def _build(num_devices=N_CORES, emb_bufs=8, psum_bufs=8, reps=1):
    f32 = mybir.dt.float32
    fp8 = mybir.dt.float8e4
    nc = bacc.Bacc("TRN2", target_bir_lowering=False, debug=False,
                   num_devices=num_devices)
    # [tile, partition, chunk, row-in-tile]: each tile's load reads one
    # contiguous DRAM extent (128 contiguous 15.9KB partition blocks), so
    # the 8 cores' concurrent streams stay HBM-row-local
    embT = nc.dram_tensor("embT", [T + 1, P, 2, NT], fp8,
                          kind="ExternalInput").ap()
    q = nc.dram_tensor("q", [P, 2, 16], fp8, kind="ExternalInput").ap()
    out_r = nc.dram_tensor("out_r", [P, 8], f32, kind="ExternalOutput").ap()
    out_i = nc.dram_tensor("out_i", [P, 8], mybir.dt.uint32,
                           kind="ExternalOutput").ap()

    with tile.TileContext(nc) as tc:
        with ExitStack() as ctx:
            const_pool = ctx.enter_context(tc.tile_pool(name="const", bufs=1))
            emb_pool = ctx.enter_context(
                tc.tile_pool(name="emb", bufs=emb_bufs))
            psum_pool = ctx.enter_context(
                tc.tile_pool(name="psum", bufs=psum_bufs, space="PSUM"))
            stage_pool = ctx.enter_context(tc.tile_pool(name="stage", bufs=2))
            res_pool = ctx.enter_context(tc.tile_pool(name="res", bufs=1))

            # [128, 2, 16]: column 0 of the last dim holds q; the padding
            # keeps the DoubleRow weight AP's chunk-dim stride at 16 bytes.
            q_sb = const_pool.tile([P, 2, 16], fp8)
            nc.sync.dma_start(out=q_sb[:], in_=q[:])

            dots = res_pool.tile([P, CC], f32)

            for t in range((T + 1) * reps):
                t = t % (T + 1)
                nt = NT if t < T else NT_L
                et = emb_pool.tile([P, 2, NT], fp8, tag="et")
                # both table loads on the SP (sync) HWDGE ring: the SP queue
                # carries nothing else, so a blocked buffer-free wait never
                # head-of-line-blocks compute-engine work
                nc.sync.dma_start(out=et[:, 0, :nt],
                                  in_=embT[t, :, 0, :nt])
                nc.sync.dma_start(out=et[:, 1, :nt],
                                  in_=embT[t, :, 1, :nt])
                stage = stage_pool.tile([1, NT], f32, tag="stage")
                # group widths: 496s, plus a 64-wide remainder on the tail
                for g0 in range(0, nt, G):
                    gw = min(G, nt - g0)
                    ps = psum_pool.tile([1, G], f32, tag="ps")
                    nc.tensor.matmul(out=ps[:, :gw], lhsT=q_sb[:, :, 0:1],
                                     rhs=et[:, :, g0:g0 + gw],
                                     start=True, stop=True,
                                     perf_mode=mybir.MatmulPerfMode.DoubleRow)
                    sl = slice(g0, g0 + gw)
                    if (g0 // G) % 2 == 0:
                        nc.scalar.copy(stage[:, sl], ps[:, :gw])
                    else:
                        nc.vector.tensor_copy(stage[:, sl], ps[:, :gw])
                c0 = t * CPT
                nc.scalar.dma_start(out=dots[:, c0:c0 + nt // P],
                                    in_=stage[:, :nt])

            rmax = res_pool.tile([P, 8], f32, tag="ep_rmax")
            ridx = res_pool.tile([P, 8], mybir.dt.uint32, tag="ep_ridx")
            nc.vector.max(out=rmax[:], in_=dots[:])
            nc.vector.max_index(out=ridx[:], in_max=rmax[:], in_values=dots[:])

            nc.sync.dma_start(out=out_r[:], in_=rmax[:])
            nc.scalar.dma_start(out=out_i[:], in_=ridx[:])

    nc.compile()
    return nc


_NC_CACHE = None


def _get_nc():
    global _NC_CACHE
    if _NC_CACHE is None:
        _NC_CACHE = _build()
    return _NC_CACHE


def make_in_maps(query_embedding, stored_embeddings):
    q = np.asarray(query_embedding, dtype=np.float32)
    emb = np.asarray(stored_embeddings, dtype=np.float32)
    qn = np.linalg.norm(q.astype(np.float64))
    qhat = (q.astype(np.float64) / (qn + EPS)).astype(np.float32)

    q_in = np.zeros((P, 2, 16), dtype=FP8)
    q_in[:, :, 0] = (qhat.reshape(2, P).T * Q_SCALE).astype(FP8)

    # [128, 2, 1M] fp8: [dim-in-chunk, chunk, row]. Cast before the
    # rearrangement so the strided copy moves 256 MB of fp8, not 1 GB of f32
    # (elementwise cast commutes with transpose).
    emb8 = emb.astype(FP8)
    embT = np.ascontiguousarray(emb8.T.reshape(2, P, N_ROWS).transpose(1, 0, 2))

    def tile_layout(cb):
        """[128, 2, ROWS_PC] -> [T+1, 128, 2, NT] (tail zero-padded)."""
        arr = np.zeros((T + 1, P, 2, NT), dtype=FP8)
        arr[:T] = cb[:, :, :T * NT].reshape(P, 2, T, NT).transpose(2, 0, 1, 3)
        arr[T, :, :, :NT_L] = cb[:, :, T * NT:]
        return arr

    in_maps = []
    for i in range(N_CORES - 1):
        sl = embT[:, :, i * ROWS_PC:(i + 1) * ROWS_PC]
        in_maps.append({"embT": tile_layout(sl), "q": q_in})
    lo = (N_CORES - 1) * ROWS_PC
    last = np.zeros((P, 2, ROWS_PC), dtype=FP8)
    last[:, :, :N_ROWS - lo] = embT[:, :, lo:]
    in_maps.append({"embT": tile_layout(last), "q": q_in})
    return in_maps


def combine(results, query_embedding, stored_embeddings):
    """Pick the global best from per-core per-partition top-8 candidates,
    rescoring every candidate with the exact f64 cosine formula."""
    q = np.asarray(query_embedding, dtype=np.float64)
    qhat = q / (np.linalg.norm(q) + EPS)
    cand = []
    for core, res in enumerate(results):
        idx = res["out_i"].astype(np.int64)
        part = np.arange(P, dtype=np.int64)[:, None]
        # full tiles: (p, c) -> (c//62)*7936 + p*62 + c%62;
        # tail tile (c >= 930): T*NT + p*47 + (c - 930)
        r_full = (idx // CPT) * NT + part * CPT + (idx % CPT)
        r_tail = T * NT + part * CPT_L + (idx - T * CPT)
        r_local = np.where(idx < T * CPT, r_full, r_tail)
        cand.append((core * ROWS_PC + r_local).ravel())
    cand = np.concatenate(cand)
    cand = np.unique(cand[(cand >= 0) & (cand < N_ROWS)])
    rows = np.asarray(stored_embeddings, dtype=np.float64)[cand]
    sims = (rows @ qhat) / (np.linalg.norm(rows, axis=1) + EPS)
    k = int(np.argmax(sims))
    return np.int32(cand[k]), np.float32(sims[k])


def kernel(query_embedding, stored_embeddings):
    nc = _get_nc()
    in_maps = make_in_maps(query_embedding, stored_embeddings)
    res = run_bass_kernel_spmd(nc, in_maps, core_ids=list(range(N_CORES)))
    return combine(res.results, query_embedding, stored_embeddings)


# revision 24
# speedup vs baseline: 1.0092x; 1.0092x over previous
"""Cosine-similarity 1-NN over 1M x 256 f32 embeddings on 8 TRN2 NeuronCores.

v3, fp8 DoubleRow streaming: the kernel is a pure HBM-bandwidth problem, so
the device-side table is stored fp8 e4m3 (quarter of f32 traffic), and the
TensorEngine's DoubleRow perf mode virtualizes the PE array to a 128x256
contraction — both 128-dim chunks of each row contract in ONE matmul at one
row/cycle. Candidate selection only needs the true argmax to survive into a
top-8-per-partition candidate set that the host rescores exactly in f64;
fp8 perturbs dots by sigma ~0.5 (at qx16 scaling) against partition-level
top-8 margins of ~30, so ranking by fp8 dots is safe (verified in emulation
vs the reference argmax: the true best ranks #1 in its partition, 75 vs 41
for the 8th-best).

Host-side prep (one-time, outside the timed NEFF): table -> [128, 2, N]
fp8 (dim d of chunk c at [d, c, row]), q -> qhat * 16 cast fp8 (scaling
centers q's entries in e4m3's dynamic range; dots scale by 16, ranking
unchanged). Cores 0-6 take 125056 rows, core 7 the rest zero-padded
(125056 = 977*128 is the minimal 128-multiple shard size).

Per-core graph, rows_pc = 125056 = 15 tiles x 7936 rows + one 6016-row
tail tile:
  - et tile [128, 2, 7936] fp8: both chunk loads on the SP (sync) HWDGE
    ring (~1 MB per DMA, 2 MB per tile). Keeping the SP queue DMA-only
    means a blocked buffer-free wait never head-of-line-blocks compute
    work (this alone was worth ~40 us vs mixing loads onto the ACT ring).
  - 16 matmuls per tile: lhsT = q3[:, :, 0:1] ([128, 2, 1] fp8), rhs =
    et[:, :, g*496:(g+1)*496] ([128, 2, 496]), perf_mode=DoubleRow ->
    dots [1, 496] f32 in PSUM, one instruction per group.
  - Evacuation alternates ACT/DVE copies into a [1, 7936] f32 stage row,
    then one ACT-ring SBUF->SBUF DMA reshapes to dots[:, t*62:(t+1)*62].
  - Epilogue: per-partition top-8 (vector.max / max_index).
Steady-state: ~100-102 us/scan sustained on HW (REPS=129 chains; vs a
TimelineSim pure-DMA floor of 89.4 us for 32.0 MB/core, i.e. ~320 GB/s
effective of the ~358 GB/s per-core HBM peak). A/B-tested alternatives
that LOST: single combined 2MB DMA per tile (109 us), tile-contiguous
DRAM layout (106 us — the 250KB-strided chunk layout spreads HBM banks
better under 8-core load), evac on GPSIMD (Q7 shares SWDGE work).

Host maps (partition p, col c) -> local row (c//62)*7936 + p*62 + c%62
for c < 930, else 15*7936 + p*47 + (c-930), and rescores all candidates
exactly.
"""
import numpy as np
import ml_dtypes
from contextlib import ExitStack

from concourse import bacc, tile, mybir
from concourse.bass_utils import run_bass_kernel_spmd

EPS = 1e-8
P = 128
D = 256
N_CORES = 8
N_ROWS = 1000000

G = 496            # dots per PSUM group (<= 512 f32 / one 2KB PSUM bank)
NG = 16            # PSUM groups per full tile
NT = G * NG        # 7936 rows per full tile (= 62 * 128)
T = 15             # full tiles per core
NT_L = 6016        # tail tile: 47*128 rows (12 groups of 496 + one of 64)
ROWS_PC = NT * T + NT_L   # 125056 = 977*128: minimal multiple of 128 with
                          # 8 * ROWS_PC >= 1M (1.5% less DMA than padding
                          # to a uniform 16th tile)
CPT = NT // P      # 62 dot columns per full tile
CPT_L = NT_L // P  # 47 dot columns in the tail tile
CC = T * CPT + CPT_L      # 977 dot columns per partition

FP8 = ml_dtypes.float8_e4m3
Q_SCALE = 16.0


def _build(num_devices=N_CORES, emb_bufs=8, psum_bufs=8, reps=1):
    f32 = mybir.dt.float32
    fp8 = mybir.dt.float8e4
    nc = bacc.Bacc("TRN2", target_bir_lowering=False, debug=False,
                   num_devices=num_devices)
    embT = nc.dram_tensor("embT", [P, 2, ROWS_PC], fp8,
                          kind="ExternalInput").ap()
    q = nc.dram_tensor("q", [P, 2, 16], fp8, kind="ExternalInput").ap()
    out_r = nc.dram_tensor("out_r", [P, 8], f32, kind="ExternalOutput").ap()
    out_i = nc.dram_tensor("out_i", [P, 8], mybir.dt.uint32,
                           kind="ExternalOutput").ap()

    with tile.TileContext(nc) as tc:
        with ExitStack() as ctx:
            const_pool = ctx.enter_context(tc.tile_pool(name="const", bufs=1))
            emb_pool = ctx.enter_context(
                tc.tile_pool(name="emb", bufs=emb_bufs))
            psum_pool = ctx.enter_context(
                tc.tile_pool(name="psum", bufs=psum_bufs, space="PSUM"))
            stage_pool = ctx.enter_context(tc.tile_pool(name="stage", bufs=2))
            res_pool = ctx.enter_context(tc.tile_pool(name="res", bufs=1))

            # [128, 2, 16]: column 0 of the last dim holds q; the padding
            # keeps the DoubleRow weight AP's chunk-dim stride at 16 bytes.
            q_sb = const_pool.tile([P, 2, 16], fp8)
            nc.sync.dma_start(out=q_sb[:], in_=q[:])

            dots = res_pool.tile([P, CC], f32)

            for t in range((T + 1) * reps):
                t = t % (T + 1)
                nt = NT if t < T else NT_L
                r0 = t * NT          # row offset (tail tile starts at T*NT)
                et = emb_pool.tile([P, 2, NT], fp8, tag="et")
                # both table loads on the SP (sync) HWDGE ring: the SP queue
                # carries nothing else, so a blocked buffer-free wait never
                # head-of-line-blocks compute-engine work
                nc.sync.dma_start(out=et[:, 0, :nt],
                                  in_=embT[:, 0, r0:r0 + nt])
                nc.sync.dma_start(out=et[:, 1, :nt],
                                  in_=embT[:, 1, r0:r0 + nt])
                stage = stage_pool.tile([1, NT], f32, tag="stage")
                # group widths: 496s, plus a 64-wide remainder on the tail
                for g0 in range(0, nt, G):
                    gw = min(G, nt - g0)
                    ps = psum_pool.tile([1, G], f32, tag="ps")
                    nc.tensor.matmul(out=ps[:, :gw], lhsT=q_sb[:, :, 0:1],
                                     rhs=et[:, :, g0:g0 + gw],
                                     start=True, stop=True,
                                     perf_mode=mybir.MatmulPerfMode.DoubleRow)
                    sl = slice(g0, g0 + gw)
                    if (g0 // G) % 2 == 0:
                        nc.scalar.copy(stage[:, sl], ps[:, :gw])
                    else:
                        nc.vector.tensor_copy(stage[:, sl], ps[:, :gw])
                c0 = t * CPT
                nc.scalar.dma_start(out=dots[:, c0:c0 + nt // P],
                                    in_=stage[:, :nt])

            rmax = res_pool.tile([P, 8], f32, tag="ep_rmax")
            ridx = res_pool.tile([P, 8], mybir.dt.uint32, tag="ep_ridx")
            nc.vector.max(out=rmax[:], in_=dots[:])
            nc.vector.max_index(out=ridx[:], in_max=rmax[:], in_values=dots[:])

            nc.sync.dma_start(out=out_r[:], in_=rmax[:])
            nc.scalar.dma_start(out=out_i[:], in_=ridx[:])

    nc.compile()
    return nc


_NC_CACHE = None


def _get_nc():
    global _NC_CACHE
    if _NC_CACHE is None:
        _NC_CACHE = _build()
    return _NC_CACHE


def make_in_maps(query_embedding, stored_embeddings):
    q = np.asarray(query_embedding, dtype=np.float32)
    emb = np.asarray(stored_embeddings, dtype=np.float32)
    qn = np.linalg.norm(q.astype(np.float64))
    qhat = (q.astype(np.float64) / (qn + EPS)).astype(np.float32)

    q_in = np.zeros((P, 2, 16), dtype=FP8)
    q_in[:, :, 0] = (qhat.reshape(2, P).T * Q_SCALE).astype(FP8)

    # [128, 2, 1M] fp8: [dim-in-chunk, chunk, row]. Cast before the
    # rearrangement so the strided copy moves 256 MB of fp8, not 1 GB of f32
    # (elementwise cast commutes with transpose).
    emb8 = emb.astype(FP8)
    embT = np.ascontiguousarray(emb8.T.reshape(2, P, N_ROWS).transpose(1, 0, 2))
    in_maps = []
    for i in range(N_CORES - 1):
        sl = embT[:, :, i * ROWS_PC:(i + 1) * ROWS_PC]
        in_maps.append({"embT": sl, "q": q_in})
    lo = (N_CORES - 1) * ROWS_PC
    last = np.zeros((P, 2, ROWS_PC), dtype=FP8)
    last[:, :, :N_ROWS - lo] = embT[:, :, lo:]
    in_maps.append({"embT": last, "q": q_in})
    return in_maps


def combine(results, query_embedding, stored_embeddings):
    """Pick the global best from per-core per-partition top-8 candidates,
    rescoring every candidate with the exact f64 cosine formula."""
    q = np.asarray(query_embedding, dtype=np.float64)
    qhat = q / (np.linalg.norm(q) + EPS)
    cand = []
    for core, res in enumerate(results):
        idx = res["out_i"].astype(np.int64)
        part = np.arange(P, dtype=np.int64)[:, None]
        # full tiles: (p, c) -> (c//62)*7936 + p*62 + c%62;
        # tail tile (c >= 930): T*NT + p*47 + (c - 930)
        r_full = (idx // CPT) * NT + part * CPT + (idx % CPT)
        r_tail = T * NT + part * CPT_L + (idx - T * CPT)
        r_local = np.where(idx < T * CPT, r_full, r_tail)
        cand.append((core * ROWS_PC + r_local).ravel())
    cand = np.concatenate(cand)
    cand = np.unique(cand[(cand >= 0) & (cand < N_ROWS)])
    rows = np.asarray(stored_embeddings, dtype=np.float64)[cand]
    sims = (rows @ qhat) / (np.linalg.norm(rows, axis=1) + EPS)
    k = int(np.argmax(sims))
    return np.int32(cand[k]), np.float32(sims[k])


def kernel(query_embedding, stored_embeddings):
    nc = _get_nc()
    in_maps = make_in_maps(query_embedding, stored_embeddings)
    res = run_bass_kernel_spmd(nc, in_maps, core_ids=list(range(N_CORES)))
    return combine(res.results, query_embedding, stored_embeddings)
